# revision 1
# baseline (speedup 1.0000x reference)
"""Multi-head attention (16 heads, d_model=2048, seq=2048, causal) on 8 trn2 cores.

Sharding: tensor-parallel over heads (2 heads/core) for QKV projection and
attention; two per-head AllToAlls redistribute the (normalized) per-head
attention outputs so each core holds all heads for a 256-row query slice;
each core then runs the full output projection for its slice and the host
concatenates the 8 slices.

Math notes:
 - Softmax without max-subtraction: scores are O(1) in fp32, exp never
   overflows.
 - K bias is dropped: it shifts every score in a query row by the same
   constant, which softmax cancels exactly.
 - V bias is folded into the output bias host-side (softmax weights sum
   to 1): bo' = bo + Wo @ bv.
 - Causality is structural: strictly-upper 128x512 blocks of the score
   matrix are skipped; diagonal-crossing blocks stream only their live
   columns into the prob/attn/denominator matmuls, and the 128-col partial
   strip is masked post-exp with a precomputed triangular bf16 mask on DVE.
 - All matmul operands bf16 (fp32 PSUM accumulation); measured end-to-end
   relative error ~4e-3.

Schedule notes:
 - Phase 1 computes V transposed (V^T) so all matmuls stream 512 columns
   with weight loads hidden, then flips it back with PE transposes
   (identity matmuls) into the phase-2 layout.
 - Phase 2 exps are batched in [128, 2x512] PSUM pairs (halves the ACT
   per-instruction overhead); at-matmuls run two pipeline steps behind
   scores in one stream per head (crossing qb boundaries), so the PE never
   drains at block boundaries. Softmax denominators accumulate on DVE in
   bf16 and are partition-reduced by a single ones-matmul per block.
 - Softmax reciprocal via DVE reciprocal_approx_fast (~0.7us vs 3.3us for
   the exact op), broadcast on gpsimd (prewarmed: first ucode launch costs
   ~7us), cc_in stores issued from the gpsimd queue (cheap dispatch).
 - Wo (8 MB) streams during phase-2 head 0 on the Sync queue. aT readbacks
   (which must wait on the AllToAll) are the only blocking Sync entries.
 - A tiny warmup AllToAll at kernel start absorbs the first-collective
   channel setup; warmed AllToAlls run ~3x faster (~9us vs ~25us).
 - Phase 3 runs head-0 contributions (j-outer, shared moving operand)
   while the second AllToAll is in flight; head-1 contributions go
   block-major so readouts/stores stagger instead of bunching at the end.
"""
import sys

sys.path.insert(0, "/opt/trn_rl_repo")

import numpy as np
import ml_dtypes

import concourse.bass as bass
import concourse.tile as tile
from concourse import mybir, bacc
import concourse.bass_utils as bass_utils
from concourse.bass_utils import run_bass_kernel_spmd


def _install_axon_profile_hook():
    """Provide antenv.axon_hooks (missing from this image) so
    run_bass_kernel_spmd(trace=True) can capture NTFF profiles via the
    axon PJRT .so, and make artifact upload failures non-fatal."""
    import types
    import ctypes
    import contextlib

    if "antenv.axon_hooks" not in sys.modules:
        mod = types.ModuleType("antenv.axon_hooks")
        _hook_holder = {"hook": None}

        def set_axon_ntff_profile_hook(h):
            _hook_holder["hook"] = h

        def get_axon_ntff_profile_hook():
            return _hook_holder["hook"]

        mod.set_axon_ntff_profile_hook = set_axon_ntff_profile_hook
        mod.get_axon_ntff_profile_hook = get_axon_ntff_profile_hook
        sys.modules["antenv.axon_hooks"] = mod

        so_path = "/opt/axon/libaxon_pjrt.so"
        try:
            lib = ctypes.CDLL(so_path)
            lib.axon_start_nrt_profile.argtypes = [
                ctypes.POINTER(ctypes.c_int64), ctypes.c_size_t]
            lib.axon_start_nrt_profile.restype = ctypes.c_int64
            lib.axon_stop_nrt_profile.argtypes = [ctypes.c_char_p]
            lib.axon_stop_nrt_profile.restype = ctypes.c_int64

            @contextlib.contextmanager
            def _hook(output_dir, device_ids):
                import jax
                jax.devices()
                if device_ids:
                    ids = (ctypes.c_int64 * len(device_ids))(*device_ids)
                    rc = lib.axon_start_nrt_profile(ids, len(device_ids))
                else:
                    rc = lib.axon_start_nrt_profile(None, 0)
                if rc != 0:
                    raise RuntimeError(f"axon_start_nrt_profile rc={rc}")
                try:
                    yield
                finally:
                    n = lib.axon_stop_nrt_profile(str(output_dir).encode())
                    print(f"profile: {n} file(s) written to {output_dir}",
                          file=sys.stderr)

            set_axon_ntff_profile_hook(_hook)
        except OSError:
            pass

    if not getattr(bass_utils.upload_artifacts, "_safe", False):
        _orig_upload = bass_utils.upload_artifacts

        def _safe_upload(tmpdir):
            try:
                return _orig_upload(tmpdir)
            except Exception:
                return str(tmpdir)

        _safe_upload._safe = True
        bass_utils.upload_artifacts = _safe_upload


_install_axon_profile_hook()

F32 = mybir.dt.float32
BF16 = mybir.dt.bfloat16
AF = mybir.ActivationFunctionType

S = 2048          # sequence length
D = 2048          # d_model
H = 16            # heads
DH = 128          # head dim
NCORES = 8
HPC = H // NCORES  # heads per core = 2
EL = HPC * DH      # local embedding slice = 256
P = 128
QROWS = S // NCORES  # output rows per core = 256
INV_SQRT_DH = float(1.0 / np.sqrt(DH))

CORE_IDS = list(range(NCORES))

_CACHE = {}

# exported for test.py: BassKernelResults of the most recent kernel() call
LAST_RESULTS = None


def _build_module():
    nc = bacc.Bacc("TRN2", target_bir_lowering=False, debug=False,
                   num_devices=NCORES)

    xT_d = nc.dram_tensor("xT", [D, S], BF16, kind="ExternalInput").ap()
    wq_d = nc.dram_tensor("wq", [D, EL], BF16, kind="ExternalInput").ap()
    wk_d = nc.dram_tensor("wk", [D, EL], BF16, kind="ExternalInput").ap()
    wv_d = nc.dram_tensor("wv", [D, EL], BF16, kind="ExternalInput").ap()
    bq_d = nc.dram_tensor("bq", [P, HPC], F32, kind="ExternalInput").ap()
    wo_d = nc.dram_tensor("wo", [D, D], BF16, kind="ExternalInput").ap()
    bo_d = nc.dram_tensor("bo", [P, D], F32, kind="ExternalInput").ap()
    tri_d = nc.dram_tensor("tri", [P, P], BF16, kind="ExternalInput").ap()
    eye_d = nc.dram_tensor("eye", [P, P], BF16, kind="ExternalInput").ap()

    # bf16 output (upcast host-side): halves the tail stores; the bf16
    # rounding adds ~2e-3 rel err against a 2e-2 gate
    out_d = nc.dram_tensor("out", [QROWS, D], BF16, kind="ExternalOutput").ap()

    # per-head collective buffers: [q-shard (dest core), dh, q-within-shard]
    cc_in = [nc.dram_tensor(f"cc_in{h}", [NCORES, P, QROWS], BF16).ap()
             for h in range(HPC)]
    cc_out = [nc.dram_tensor(f"cc_out{h}", [NCORES, P, QROWS], BF16).ap()
              for h in range(HPC)]
    warm_in = nc.dram_tensor("warm_in", [NCORES, 16], BF16).ap()
    warm_out = nc.dram_tensor("warm_out", [NCORES, 16], BF16).ap()

    with tile.TileContext(nc, num_cores=NCORES) as tc:
        with (
            tc.tile_pool(name="const", bufs=1) as cpool,
            tc.tile_pool(name="qkv", bufs=1) as qkv_pool,
        ):
            ones_bf = cpool.tile([P, 1], BF16, name="ones_bf")
            nc.vector.memset(ones_bf[:], 1.0)
            tri_t = cpool.tile([P, P], BF16, name="tri_t")
            eye_t = cpool.tile([P, P], BF16, name="eye_t")
            bq_t = cpool.tile([P, HPC], F32, name="bq_t")

            # warm the collective channel while phase 1 runs
            nc.gpsimd.collective_compute(
                "AllToAll", mybir.AluOpType.bypass,
                replica_groups=[CORE_IDS],
                ins=[warm_in[:]], outs=[warm_out[:]])
            # warm the gpsimd broadcast ucode (first launch pays ~7us)
            wsrc = cpool.tile([1, 16], F32, name="wsrc")
            nc.vector.memset(wsrc[:], 1.0)
            wdst = cpool.tile([P, 16], F32, name="wdst")
            nc.gpsimd.partition_broadcast(wdst[:], wsrc[:])

            # per-head Q^T/K^T [dh, s] (bf16, Q pre-scaled by 1/sqrt(dh)) and
            # V [s, head, k-chunk, dh] (bf16) resident in SBUF
            QT = [qkv_pool.tile([P, S], BF16, name=f"QT{h}") for h in range(HPC)]
            KT = [qkv_pool.tile([P, S], BF16, name=f"KT{h}") for h in range(HPC)]
            V_t = qkv_pool.tile([P, HPC, S // P, DH], BF16, name="V_t")

            # output-projection weights + bias + attn readback, loaded later
            p3 = tc.alloc_tile_pool(name="p3", bufs=1)
            wo_t = p3.tile([P, H, D], BF16, name="wo_t")
            bo_t = p3.tile([P, D], F32, name="bo_t")
            aT = [p3.tile([P, NCORES, QROWS], BF16, name=f"aT{h}")
                  for h in range(HPC)]
            nc.scalar.dma_start(bo_t[:], bo_d[:])

            # ---------------- Phase 1: QKV projection ----------------
            # V is computed transposed (V^T [e, s]) so every matmul streams
            # 512 columns and all weight loads hide under the streams; the
            # XBAR DMA transpose then lays V out [s, e] for phase 2.
            with (
                tc.tile_pool(name="w", bufs=1) as wpool,
                tc.tile_pool(name="vsb", bufs=2) as vsb_pool,
                tc.tile_pool(name="xt", bufs=6) as xt_pool,
                tc.tile_pool(name="ps_qk", bufs=1, space="PSUM") as ps_qk,
                tc.tile_pool(name="ps_vt", bufs=1, space="PSUM") as ps_vt,
                tc.tile_pool(name="ps_tr", bufs=2, space="PSUM") as ps_tr,
            ):
                wq_t = wpool.tile([P, D // P, EL], BF16, name="wq_t")
                wk_t = wpool.tile([P, D // P, EL], BF16, name="wk_t")
                wv_t = wpool.tile([P, D // P, EL], BF16, name="wv_t")

                W_PAIRS = ((wv_t, wv_d), (wk_t, wk_d), (wq_t, wq_d))

                def load_w_chunk(c4):
                    for w_t, w_d in W_PAIRS:
                        dsl = slice(c4 * (D // P // 4),
                                    (c4 + 1) * (D // P // 4))
                        rsl = slice(c4 * (D // 4), (c4 + 1) * (D // 4))
                        nc.sync.dma_start(
                            w_t[:, dsl, :],
                            w_d[rsl, :].rearrange("(dc p) e -> p dc e", p=P))

                # 128-row head pieces of each weight first, on the scalar
                # queue so they stream in parallel with x on the sync queue:
                # the first matmul starts after ~250KB of DMA instead of
                # ~1.3MB, and the (ramp-speed) first group never starves
                for w_t, w_d in W_PAIRS:
                    nc.scalar.dma_start(w_t[:, 0, :], w_d[0:P, :])
                vt_sb_prev = [None]

                def emit_v_transposes(sbi_prev):
                    # V^T -> V via PE transpose (identity matmul), bf16 PSUM,
                    # then DVE copies into the phase-2 V layout
                    vt_sb = vt_sb_prev[0]
                    for hl in range(HPC):
                        for c in range(4):
                            tr = ps_tr.tile([P, P], BF16, name="tr")
                            nc.tensor.transpose(
                                tr[:], vt_sb[:, hl, c * P:(c + 1) * P],
                                eye_t[:])
                            nc.vector.tensor_scalar_mul(
                                V_t[:, hl, sbi_prev * 4 + c, :], tr[:], 1.0)

                for sbi in range(S // 512):
                    q0 = ps_qk.tile([P, 512], F32, name="q0")
                    q1 = ps_qk.tile([P, 512], F32, name="q1")
                    k0 = ps_qk.tile([P, 512], F32, name="k0")
                    k1 = ps_qk.tile([P, 512], F32, name="k1")
                    vt0 = ps_vt.tile([P, 512], F32, name="vt0")
                    vt1 = ps_vt.tile([P, 512], F32, name="vt1")
                    # one DMA brings 4 d-chunks of x (fewer, larger issues)
                    for dc4 in range(D // P // 4):
                        xt = xt_pool.tile([P, 4, 512], BF16, name="xt")
                        if sbi == 0 and dc4 == 0:
                            # fine-grained 128-row interleave of x slices and
                            # weight rows: each d-chunk's operands arrive just
                            # in time, so the slow (p-state ramp) first group
                            # never starves
                            for i4 in range(4):
                                nc.sync.dma_start(
                                    xt[:, i4, :],
                                    xT_d[i4 * P:(i4 + 1) * P, 0:512])
                            for w_t, w_d in W_PAIRS:
                                nc.scalar.dma_start(
                                    w_t[:, 1:4, :],
                                    w_d[P:4 * P, :]
                                    .rearrange("(dc p) e -> p dc e", p=P))
                            # constants aren't needed until later; dispatch
                            # behind the startup-critical weight pieces
                            nc.scalar.dma_start(tri_t[:], tri_d[:])
                            nc.scalar.dma_start(eye_t[:], eye_d[:])
                            nc.scalar.dma_start(bq_t[:], bq_d[:])
                        else:
                            nc.sync.dma_start(
                                xt[:],
                                xT_d[dc4 * 4 * P:(dc4 + 1) * 4 * P,
                                     sbi * 512:(sbi + 1) * 512]
                                .rearrange("(i p) s -> p i s", p=P))
                        if sbi == 0 and dc4 < 3:
                            # stream the remaining weight quarters just ahead
                            # of the d-chunks that need them
                            load_w_chunk(dc4 + 1)
                        for i in range(4):
                            dc = dc4 * 4 + i
                            st, sp = dc == 0, dc == (D // P - 1)
                            xti = xt[:, i, :]
                            # vt first (DVE drains those banks fastest at the
                            # sbi boundary), then k (also DVE), q last (ACT)
                            nc.tensor.matmul(vt0[:], wv_t[:, dc, 0:P], xti,
                                             start=st, stop=sp)
                            nc.tensor.matmul(vt1[:], wv_t[:, dc, P:EL], xti,
                                             start=st, stop=sp)
                            nc.tensor.matmul(k0[:], wk_t[:, dc, 0:P], xti,
                                             start=st, stop=sp)
                            nc.tensor.matmul(k1[:], wk_t[:, dc, P:EL], xti,
                                             start=st, stop=sp)
                            nc.tensor.matmul(q0[:], wq_t[:, dc, 0:P], xti,
                                             start=st, stop=sp)
                            nc.tensor.matmul(q1[:], wq_t[:, dc, P:EL], xti,
                                             start=st, stop=sp)
                        if dc4 == 1 and sbi > 0:
                            emit_v_transposes(sbi - 1)
                    s_sl = slice(sbi * 512, (sbi + 1) * 512)
                    vt_sb = vsb_pool.tile([P, HPC, 512], BF16, name="vt_sb")
                    nc.vector.tensor_scalar_mul(vt_sb[:, 0, :], vt0[:], 1.0)
                    nc.vector.tensor_scalar_mul(vt_sb[:, 1, :], vt1[:], 1.0)
                    nc.vector.tensor_scalar_mul(KT[0][:, s_sl], k0[:], 1.0)
                    nc.vector.tensor_scalar_mul(KT[1][:, s_sl], k1[:], 1.0)
                    vt_sb_prev[0] = vt_sb
                    nc.scalar.activation(QT[0][:, s_sl], q0[:], AF.Identity,
                                         bias=bq_t[:, 0:1], scale=INV_SQRT_DH)
                    nc.scalar.activation(QT[1][:, s_sl], q1[:], AF.Identity,
                                         bias=bq_t[:, 1:2], scale=INV_SQRT_DH)
                emit_v_transposes(3)

            # ---------------- Phase 2: attention per head ----------------
            with (
                tc.tile_pool(name="pt", bufs=4) as pt_pool,
                tc.tile_pool(name="dacc", bufs=2) as dacc_pool,
                tc.tile_pool(name="nrm", bufs=2) as nrm,
                tc.tile_pool(name="ps_s", bufs=2, space="PSUM") as ps_s,
                tc.tile_pool(name="ps_at", bufs=2, space="PSUM") as ps_at,
                tc.tile_pool(name="ps_den", bufs=2, space="PSUM") as ps_den,
            ):
                nc.scalar.dma_start(bo_t[:], bo_d[:])
                for h in range(HPC):
                    # one software-pipelined stream over all (qb, pair) steps
                    # of this head: at-matmuls run two steps behind scores, so
                    # the PE never drains at qb boundaries
                    steps = [(qb, u) for qb in range(S // 512)
                             for u in range(2 * (qb + 1))]
                    at_tiles = {}
                    dacc_tiles = {}
                    pending = {}

                    def emit_at(qb, u):
                        pt, offs = pending.pop((qb, u))
                        nkc = 4 * (qb + 1)
                        if qb not in at_tiles:
                            at_tiles[qb] = ps_at.tile([P, 512], F32,
                                                      name="at_ps")
                        at_ps = at_tiles[qb]
                        for j in (0, 1):
                            kc = 2 * u + j
                            off = offs[j]
                            st, sp = kc == 0, kc == nkc - 1
                            nc.tensor.matmul(
                                at_ps[:, off:512], V_t[:, h, kc, :],
                                pt[:, j, off:512], start=st, stop=sp)

                    def finish_qb(qb):
                        # partition-reduce the DVE denominator accumulator,
                        # normalize, and ship this qb's two dest slices
                        den_ps = ps_den.tile([1, 512], F32, name="den_ps")
                        nc.tensor.matmul(den_ps[0:1, :], ones_bf[:],
                                         dacc_tiles.pop(qb)[:],
                                         start=True, stop=True)
                        rd = nrm.tile([1, 512], F32, name="rd")
                        nc.vector.reciprocal_approx_fast(rd[:], den_ps[0:1, :])
                        rb = nrm.tile([P, 512], F32, name="rb")
                        nc.gpsimd.partition_broadcast(rb[:], rd[:])
                        at_bf = nrm.tile([P, 512], BF16, name="at_bf")
                        nc.vector.tensor_mul(at_bf[:], at_tiles.pop(qb)[:],
                                             rb[:])
                        for i in range(2):
                            nc.gpsimd.dma_start(
                                cc_in[h][2 * qb + i, :, :],
                                at_bf[:, i * QROWS:(i + 1) * QROWS])
                        if h == 0:
                            # stream Wo during head 0 on the (otherwise idle)
                            # Sync queue; done well before phase 3 needs it
                            for t in range(2):
                                g = 2 * qb + t
                                nc.sync.dma_start(
                                    wo_t[:, 2 * g:2 * g + 2, :],
                                    wo_d[g * 256:(g + 1) * 256, :]
                                    .rearrange("(ec p) f -> p ec f", p=P))

                    for si, (qb, u) in enumerate(steps):
                        npairs = 2 * (qb + 1)
                        s_pair = ps_s.tile([P, 2, 512], F32, name="s_pair")
                        pt = pt_pool.tile([P, 2, 512], BF16, name="pt")
                        offs = []
                        for j in (0, 1):
                            kc = 2 * u + j
                            off = max(0, kc * P - qb * 512)
                            offs.append(off)
                            nc.tensor.matmul(
                                s_pair[:, j, off:512],
                                KT[h][:, kc * P:(kc + 1) * P],
                                QT[h][:, qb * 512 + off:(qb + 1) * 512],
                                start=True, stop=True)
                        if 2 * u >= 4 * qb:
                            # diagonal pair: exp the live columns, then mask
                            # the 128-col partial strips
                            if h == 0 and qb == 0:
                                # first-ever use of these PSUM banks: the
                                # dead columns are uninitialized, so exp
                                # each half's live window separately
                                for j in (0, 1):
                                    off = offs[j]
                                    nc.scalar.activation(
                                        pt[:, j, off:512],
                                        s_pair[:, j, off:512], AF.Exp)
                            else:
                                # one call over the union window; the extra
                                # columns hold stale (but initialized) PSUM
                                # and are never read downstream
                                off = offs[0]
                                nc.scalar.activation(
                                    pt[:, :, off:512],
                                    s_pair[:, :, off:512], AF.Exp)
                            for j in (0, 1):
                                off = offs[j]
                                nc.vector.tensor_mul(
                                    pt[:, j, off:off + P],
                                    pt[:, j, off:off + P], tri_t[:])
                        else:
                            nc.scalar.activation(pt[:, :, :],
                                                 s_pair[:, :, :], AF.Exp)
                        # denominator partial sums on DVE (bf16, 2 el/cyc);
                        # u==0/j==0 is always unrestricted and initializes
                        # the full accumulator
                        for j in (0, 1):
                            off = offs[j]
                            if u == 0 and j == 0:
                                dacc_tiles[qb] = dacc_pool.tile(
                                    [P, 512], BF16, name="dacc")
                                nc.vector.tensor_scalar_mul(
                                    dacc_tiles[qb][:, :], pt[:, 0, :], 1.0)
                            else:
                                nc.vector.tensor_add(
                                    dacc_tiles[qb][:, off:512],
                                    dacc_tiles[qb][:, off:512],
                                    pt[:, j, off:512])
                        pending[(qb, u)] = (pt, offs)
                        if si >= 2:
                            pqb, pu = steps[si - 2]
                            emit_at(pqb, pu)
                            if pu == 2 * (pqb + 1) - 1:
                                finish_qb(pqb)
                    for qb, u in steps[-2:]:
                        emit_at(qb, u)
                        if u == 2 * (qb + 1) - 1:
                            finish_qb(qb)
                    # redistribute this head's outputs; trigger fires as soon
                    # as the last cc_in DMA lands (gpsimd queue is near-empty)
                    nc.gpsimd.collective_compute(
                        "AllToAll", mybir.AluOpType.bypass,
                        replica_groups=[CORE_IDS],
                        ins=[cc_in[h][:]], outs=[cc_out[h][:]])
                    # readback waits on the collective; Sync has nothing
                    # time-critical after this point. Piecewise (per source
                    # pair) so phase 3's first matmuls start ~2us sooner
                    for jp in range(NCORES // 2):
                        nc.sync.dma_start(
                            aT[h][:, 2 * jp:2 * jp + 2, :],
                            cc_out[h][2 * jp:2 * jp + 2, :, :]
                            .rearrange("j p q -> p j q"))

            # ---------------- Phase 3: output projection ----------------
            with (
                tc.tile_pool(name="osb", bufs=3) as osb,
                tc.tile_pool(name="ps_o", bufs=1, space="PSUM") as ps_o,
            ):
                # cc_out[h][j, p, q] = attn^T for global head (2j+h), own q slice
                # all 8 (qc, fb) groups live in 8 PSUM banks at once; all
                # head-0 contributions (available after the first AllToAll)
                # run first, overlapping the second AllToAll
                blocks = [(qc, fb) for qc in range(QROWS // P)
                          for fb in range(D // 512)]
                # lazy PSUM allocation: each block's WAR wait on the phase-2
                # banks lands at its first matmul, so blocks on early-freed
                # banks start immediately
                o_ps = {}
                # head 0 (available first): j-outer/qc-inner so consecutive
                # matmuls share the moving operand (same wo_t slice)
                for j in range(NCORES):
                    for fb in range(D // 512):
                        for qc in range(QROWS // P):
                            if (qc, fb) not in o_ps:
                                o_ps[(qc, fb)] = ps_o.tile(
                                    [P, 512], F32,
                                    name=f"o_ps_{qc}_{fb}")
                            nc.tensor.matmul(
                                o_ps[(qc, fb)][:],
                                aT[0][:, j, qc * P:(qc + 1) * P],
                                wo_t[:, 2 * j, fb * 512:(fb + 1) * 512],
                                start=(j == 0), stop=False)
                # head 1: block-major so each block's accumulation finishes
                # staggered and its readout/store overlaps the next block
                for qc, fb in blocks:
                    for j in range(NCORES):
                        nc.tensor.matmul(
                            o_ps[(qc, fb)][:],
                            aT[1][:, j, qc * P:(qc + 1) * P],
                            wo_t[:, 2 * j + 1, fb * 512:(fb + 1) * 512],
                            start=False, stop=(j == NCORES - 1))
                    o_sb = osb.tile([P, 512], BF16, name="o_sb")
                    nc.vector.tensor_add(o_sb[:], o_ps[(qc, fb)][:],
                                         bo_t[:, fb * 512:(fb + 1) * 512])
                    nc.sync.dma_start(
                        out_d[qc * P:(qc + 1) * P, fb * 512:(fb + 1) * 512],
                        o_sb[:])
            p3.release()

    nc.finalize()
    return nc


def kernel(x, mask, Wq, bq, Wk, bk, Wv, bv, Wo, bo):
    """Full-input MHA forward. Returns the full (2048, 2048) fp32 output.

    The mask input is assumed to be the strictly-upper-triangular causal mask
    the reference generates; causality is applied structurally on-device.
    """
    global LAST_RESULTS
    if "nc" not in _CACHE:
        _CACHE["nc"] = _build_module()
    nc = _CACHE["nc"]

    x = np.asarray(x, dtype=np.float32)
    Wq = np.asarray(Wq, dtype=np.float32)
    Wk = np.asarray(Wk, dtype=np.float32)
    Wv = np.asarray(Wv, dtype=np.float32)
    Wo = np.asarray(Wo, dtype=np.float32)
    bq = np.asarray(bq, dtype=np.float32)
    bv = np.asarray(bv, dtype=np.float32)
    bo = np.asarray(bo, dtype=np.float32)

    bf = ml_dtypes.bfloat16
    xT = np.ascontiguousarray(x.T).astype(bf)
    woT_bf = np.ascontiguousarray(Wo.T).astype(bf)
    # V bias folded into the output bias (softmax weights sum to 1);
    # K bias dropped entirely (softmax-invariant per-query shift)
    bo_full = bo + Wo @ bv
    bo_b = np.ascontiguousarray(np.broadcast_to(bo_full, (P, D)))
    tri = np.ascontiguousarray(np.triu(np.ones((P, P), np.float32))).astype(bf)
    eye = np.ascontiguousarray(np.eye(P, dtype=np.float32)).astype(bf)

    in_maps = []
    for c in range(NCORES):
        e_sl = slice(c * EL, (c + 1) * EL)
        in_maps.append({
            "xT": xT,
            "wq": np.ascontiguousarray(Wq[e_sl, :].T).astype(bf),
            "wk": np.ascontiguousarray(Wk[e_sl, :].T).astype(bf),
            "wv": np.ascontiguousarray(Wv[e_sl, :].T).astype(bf),
            # bias layout [dh, head]; Q bias pre-scaled by 1/sqrt(dh)
            "bq": np.ascontiguousarray((bq[e_sl] * INV_SQRT_DH).reshape(HPC, P).T),
            "wo": woT_bf,
            "bo": bo_b,
            "tri": tri,
            "eye": eye,
        })

    res = run_bass_kernel_spmd(nc, in_maps, CORE_IDS)
    LAST_RESULTS = res
    return np.concatenate(
        [np.asarray(res.results[c]["out"]).astype(np.float32)
         for c in range(NCORES)], axis=0)

